# revision 1
# baseline (speedup 1.0000x reference)
"""Trainium2 Bass kernel for the k-mer transformer problem.

Semantics (k=3, one-hot 3-mer filters over 4 bases):
    z[b, c, l] = relu(x[b,0,l,d0] + x[b,0,l+1,d1] + x[b,0,l+2,d2] - 2)
      where c = 16*d0 + 4*d1 + d2,  l in [0, 99999)
    out[b, 0, r*33333 + q, c] = z[b, c, 3q + r]      (mod-3 interleave)

Strategy: pure data parallel (batch elem b -> NeuronCore b). Per core the
output (25.6 MB) is produced directly in the permuted order so every store
is a dense contiguous DMA. Channel expansion is done on-chip with two
vector-engine tensor_tensor adds using step-0 (broadcast) access patterns,
and relu(x-2) is fused into one scalar-engine activation pass.

Per-partition layout: partition p owns q in [261*p, 261*(p+1)), i.e. x rows
[783*p, 783*p + 785). The host stages x as a [128, 3160] f32 array whose
row p is x.flat[3132*p : 3132*p + 3160] (zero padded past the end).
"""

import sys

import numpy as np

sys.path.insert(0, "/opt/trn_rl_repo")

import concourse.bacc as bacc  # noqa: E402
import concourse.mybir as mybir  # noqa: E402
from concourse.bass_utils import run_bass_kernel_spmd  # noqa: E402
from concourse.tile import TileContext  # noqa: E402

P = 128  # SBUF partitions
QP = 261  # q-positions per partition (padded: 128*261 = 33408 >= 33333)
Q = 33333  # valid q-positions per phase (99999 / 3)
CHUNKS = [(0, 131), (131, 130)]  # (q-local start, len): 261 positions, and
# 1 input + 3*2 output DMAs = 7 <= 8 HWDGE sem lanes (9+ DMAs would add a
# lane-reuse wait -> >1 wait per DMA -> walrus "Too many sync wait commands")
XW = 3160  # staged f32 per partition (>= 12*174 + 8 + 8 + 12*87 = 3148)
XSTRIDE = 3132  # f32 advance per partition (783 rows * 4 ch)
L = 100001
N_CORES = 8

_CACHE = {}


def _build_bass():
    # Bacc (not raw Bass): its finalize() runs generate_event_semaphores,
    # which splits multi-sem waits (HW allows at most 1 wait per inst).
    nc = bacc.Bacc()
    f32 = mybir.dt.float32
    add = mybir.AluOpType.add
    relu = mybir.ActivationFunctionType.Relu

    x_d = nc.declare_dram_parameter("x", [P, XW], f32, isOutput=False)
    y_d = nc.declare_dram_parameter("y", [3, P, QP * 64], f32, isOutput=True)

    with TileContext(nc) as tc:
        with (
            tc.tile_pool(name="xp", bufs=1) as xp,
            tc.tile_pool(name="t1p", bufs=2) as t1p,
            tc.tile_pool(name="t2p", bufs=2) as t2p,
            tc.tile_pool(name="op_", bufs=2) as op_,
        ):
            x_sb = xp.tile([P, XW], f32)
            nc.sync.dma_start(out=x_sb, in_=x_d[:])
            bias_sb = xp.tile([P, 1], f32, tag="bias")
            nc.vector.memset(bias_sb, -2.0)
            for r in range(3):
                for g0, G in CHUNKS:
                    base = 12 * g0 + 4 * r
                    # A[p, t, d0] broadcast over d1: [[12,G],[1,4],[0,4]]
                    a_ap = (
                        x_sb[:, base : base + 12 * G]
                        .rearrange("p (t u) -> p t u", u=12)[:, :, 0:4]
                        .broadcast_to([P, G, 4, 4])
                    )
                    # B[p, t, d1] tiled over d0: [[12,G],[0,4],[1,4]]
                    b_ap = (
                        x_sb[:, base + 4 : base + 4 + 12 * G]
                        .rearrange("p (t u) -> p t u", u=12)[:, :, 0:4]
                        .unsqueeze(2)
                        .broadcast_to([P, G, 4, 4])
                    )
                    t1 = t1p.tile([P, G * 16], f32, tag="t1")
                    nc.vector.tensor_tensor(
                        t1.rearrange("p (t a b) -> p t a b", a=4, b=4),
                        a_ap,
                        b_ap,
                        add,
                    )
                    # T1[p, t, e] broadcast over d2: [[16,G],[1,16],[0,4]]
                    t1_b = t1.rearrange("p (t e) -> p t e", e=16).broadcast_to(
                        [P, G, 16, 4]
                    )
                    # C[p, t, d2] tiled over e: [[12,G],[0,16],[1,4]]
                    c_ap = (
                        x_sb[:, base + 8 : base + 8 + 12 * G]
                        .rearrange("p (t u) -> p t u", u=12)[:, :, 0:4]
                        .unsqueeze(2)
                        .broadcast_to([P, G, 16, 4])
                    )
                    t2 = t2p.tile([P, G * 64], f32, tag="t2")
                    nc.vector.tensor_tensor(
                        t2.rearrange("p (t e b) -> p t e b", e=16, b=4),
                        t1_b,
                        c_ap,
                        add,
                    )
                    # o = relu(t2 - 2) on the scalar engine; separate tile so
                    # the store DMA depends only on ACT (HWDGE allows 1 wait)
                    o = op_.tile([P, G * 64], f32, tag="o")
                    nc.scalar.activation(o, t2, relu, bias=bias_sb)
                    # alternate stores between the two HWDGE rings (SP and
                    # ACT queues) so per-DMA completion latencies overlap
                    dma_eng = nc.scalar if (r + g0) % 2 else nc.sync
                    dma_eng.dma_start(
                        out=y_d[r, :, g0 * 64 : (g0 + G) * 64], in_=o
                    )
    return nc


def _stage_inputs(x):
    """x: [8, 1, L, 4] f32 -> list of per-core {'x': [P, XW] f32}."""
    need = XSTRIDE * (P - 1) + XW
    in_maps = []
    for b in range(x.shape[0]):
        xf = np.zeros(need, dtype=np.float32)
        xf[: L * 4] = x[b, 0].ravel()
        xs = np.lib.stride_tricks.as_strided(
            xf, shape=(P, XW), strides=(XSTRIDE * 4, 4)
        )
        in_maps.append({"x": np.ascontiguousarray(xs)})
    return in_maps


def _gather_output(results):
    out = np.empty((len(results), 1, 3 * Q, 64), dtype=np.float32)
    for b, res in enumerate(results):
        y = res["y"].reshape(3, P * QP, 64)[:, :Q, :]
        out[b, 0] = y.reshape(3 * Q, 64)
    return out


def _built_and_finalized():
    if "nc" not in _CACHE:
        nc = _build_bass()
        # run_bass_via_pjrt never finalizes; Bacc.finalize runs the register
        # allocation + sync-wait legalization passes walrus requires.
        nc.finalize()
        _CACHE["nc"] = nc
    return _CACHE["nc"]


def run(x, trace=False):
    nc = _built_and_finalized()
    in_maps = _stage_inputs(np.asarray(x, dtype=np.float32))
    bkr = run_bass_kernel_spmd(nc, in_maps, list(range(N_CORES)), trace=trace)
    return _gather_output(bkr.results), bkr


def kernel(x, W=None):
    out, _ = run(x, trace=False)
    return out

